# revision 10
# baseline (speedup 1.0000x reference)
"""Trainium2 Bass kernel for a causal single-head attention block.

Problem: y = softmax(mask(Q K^T / sqrt(H))) V with
  x  [B=4, T=4096, C=1024] f32,  Wq/Wk/Wv [C, H=64] f32.

Sharding (8 NeuronCores): data-parallel over B across core pairs;
within a pair, the T dimension is split by interleaved 512-row tiles
(rank r owns global q-tiles {2s+r}) so the causal work is balanced.
Each core computes Q/K/V for its own 2048 rows, the pair exchanges
K^T and V via an AllGather, and each core runs a flash-attention style
kc-outer loop over its own query rows.

The graph is identical on all 8 cores (SPMD); all rank-dependent
causality is delivered via input *data* (a sliding causal mask sheet).

Layout notes:
 - The host pre-transposes x per core to x^T [C, 2048] bf16 so the
   projections can contract over C on the partition dimension without
   any on-chip transpose of x.
 - Projections produce Q^T/K^T/V^T [64, T] directly (H on partitions),
   which is exactly the operand layout the S^T matmul wants.
 - S^T tiles are [128 k, 512 q]; exp has no running max (logits here
   are ~N(0,1), |s| < ~7, so exp is safe in f32) and the row-sum is
   folded into the PV matmul via a ones-column appended to V.
"""

import numpy as np
import ml_dtypes

import concourse.bass as bass
import concourse.bacc as bacc
import concourse.mybir as mybir
from concourse.tile import TileContext
from concourse.bass_utils import run_bass_kernel_spmd

BF16 = mybir.dt.bfloat16
F32 = mybir.dt.float32
bf16 = ml_dtypes.bfloat16

B, T, C, H = 4, 4096, 1024, 64
N_CORES = 8
TOWN = 2048          # rows owned per core
NSLOT = 4            # q-tiles of 512 rows per core
QT512 = 512
KC = 128             # k-chunk rows
NKC = T // KC        # 32 global k-chunks
W_SLOT = [8, 16, 24, 32]   # uniform kc-window per slot
V_FLAT = TOWN * H    # 131072 elements of V shard in the bounce
CC_IN = 128 * 3072   # staging tile, flattened
MASK_W = 896 + 512   # causal mask sheet width


def build_bass():
    nc = bacc.Bacc(
        "TRN2",
        target_bir_lowering=False,
        debug=False,
        enable_asserts=False,
        num_devices=N_CORES,
    )

    xT = nc.declare_dram_parameter("xT", [C, TOWN], BF16, isOutput=False)
    wq = nc.declare_dram_parameter("wq", [C, H], BF16, isOutput=False)
    wk = nc.declare_dram_parameter("wk", [C, H], BF16, isOutput=False)
    wv = nc.declare_dram_parameter("wv", [C, H], BF16, isOutput=False)
    ident = nc.declare_dram_parameter("ident", [H, H], BF16, isOutput=False)
    mask = nc.declare_dram_parameter("mask", [128, MASK_W], BF16, isOutput=False)
    out = nc.declare_dram_parameter("out", [H, TOWN], F32, isOutput=True)

    cc_in = nc.dram_tensor("cc_in", [CC_IN], BF16)
    cc_out = nc.dram_tensor("cc_out", [2 * CC_IN], BF16)

    with TileContext(nc) as tc:
        with (
            tc.tile_pool(name="persist", bufs=1) as pp,
            tc.tile_pool(name="work", bufs=3) as wp,
        ):
            # ---- persistent SBUF tensors ----
            xT_sb = pp.tile([128, 8, TOWN], BF16, tag="xT")
            wq_sb = pp.tile([128, 8, H], BF16, tag="wq")
            wk_sb = pp.tile([128, 8, H], BF16, tag="wk")
            wv_sb = pp.tile([128, 8, H], BF16, tag="wv")
            id_sb = pp.tile([H, H], BF16, tag="ident")
            mask_sb = pp.tile([128, MASK_W], BF16, tag="mask")
            qT_sb = pp.tile([H, TOWN], BF16, tag="qT")
            vT_own = pp.tile([H, TOWN], BF16, tag="vTown")
            stage = pp.tile([128, 3072], BF16, tag="stage")
            # rank-blocked columns: [rank0 2048 | rank1 2048]
            kT_g = pp.tile([H, T], BF16, tag="kTg")
            # V rank-blocked [128, kcb, 65]; col 64 of each chunk is ones
            vaug = pp.tile([128, NKC, H + 1], BF16, tag="vaug")
            ones_sb = pp.tile([1, H], F32, tag="ones")
            yT_sb = pp.tile([H, TOWN], F32, tag="yT")

            # ---- loads ----
            nc.sync.dma_start(
                out=xT_sb[:], in_=xT[:].rearrange("(cc p) t -> p cc t", p=128)
            )
            nc.sync.dma_start(
                out=wq_sb[:], in_=wq[:].rearrange("(cc p) h -> p cc h", p=128)
            )
            nc.sync.dma_start(
                out=wk_sb[:], in_=wk[:].rearrange("(cc p) h -> p cc h", p=128)
            )
            nc.sync.dma_start(
                out=wv_sb[:], in_=wv[:].rearrange("(cc p) h -> p cc h", p=128)
            )
            nc.sync.dma_start(out=id_sb[:], in_=ident[:])
            nc.sync.dma_start(out=mask_sb[:], in_=mask[:])
            nc.vector.memset(ones_sb[:], 1.0)
            nc.vector.memset(vaug[:, :, H : H + 1], 1.0)

            # ---- projections: Q^T, K^T, V^T for own rows ----
            with tc.tile_pool(name="proj_ps", bufs=2, space="PSUM") as proj_ps:
                for tt in range(NSLOT):
                    sl = slice(tt * QT512, (tt + 1) * QT512)
                    k_dst = stage[0:H, 1024 + tt * QT512 : 1024 + (tt + 1) * QT512]
                    for w_sb, dst in ((wq_sb, qT_sb[:, sl]), (wk_sb, k_dst), (wv_sb, vT_own[:, sl])):
                        ps = proj_ps.tile([H, QT512], F32, tag="proj")
                        for cc in range(8):
                            nc.tensor.matmul(
                                ps[:],
                                w_sb[:, cc, :],
                                xT_sb[:, cc, sl],
                                start=(cc == 0),
                                stop=(cc == 7),
                            )
                        nc.vector.tensor_copy(dst, ps[:])

                # V natural (for the PV stationary operand): PE transpose
                for tcn in range(16):
                    pst = proj_ps.tile([128, H], BF16, tag="vt")
                    nc.tensor.transpose(
                        pst[:], vT_own[:, tcn * 128 : (tcn + 1) * 128], id_sb[:]
                    )
                    nc.vector.tensor_copy(stage[:, tcn * H : (tcn + 1) * H], pst[:])

            # ---- pair exchange of K^T and V ----
            nc.gpsimd.dma_start(
                out=cc_in[:].rearrange("(p c) -> p c", p=128),
                in_=stage[:],
            )
            nc.gpsimd.collective_compute(
                "AllGather",
                mybir.AluOpType.bypass,
                replica_groups=[[2 * i, 2 * i + 1] for i in range(N_CORES // 2)],
                ins=[cc_in[:]],
                outs=[cc_out[:]],
            )
            # gathered layout: block gp (=rank), then the [128, 3072] stage rows
            blk = cc_out[:].rearrange("(gp q c) -> gp q c", gp=2, q=128)
            # kT_g rank-blocked: cols [gp*2048 + t_rank]
            nc.gpsimd.dma_start(
                out=kT_g[:].rearrange("p (gp t) -> p gp t", gp=2),
                in_=blk[:, 0:H, 1024:3072].rearrange("gp p t -> p gp t"),
            )
            # vaug rank-blocked: chunk kcb = gp*16 + tc
            for gp in range(2):
                nc.gpsimd.dma_start(
                    out=vaug[:, gp * 16 : (gp + 1) * 16, 0:H],
                    in_=blk[gp, :, 0:1024].rearrange("p (tc h) -> p tc h", h=H),
                )

            # ---- attention: kc-outer flash loop ----
            with (
                tc.tile_pool(name="swide", bufs=2, space="PSUM") as sp,
                tc.tile_pool(name="yacc", bufs=1, space="PSUM") as yp,
            ):
                y_acc = [
                    yp.tile([128, QT512], F32, tag=f"y{s}", name=f"y_acc{s}")
                    for s in range(NSLOT)
                ]

                for kc in range(NKC):
                    g = kc // 4
                    kcol = (g % 2) * 2048 + (g // 2) * QT512 + (kc % 4) * KC
                    kcb = (g % 2) * 16 + (g // 2) * 4 + kc % 4
                    smin = kc // 8
                    slots = list(range(smin, NSLOT))
                    for gi in range(0, len(slots), 2):
                        grp = slots[gi : gi + 2]
                        fd = QT512 * len(grp)
                        sw = sp.tile([128, 1024], F32, tag="swide")
                        for i, s in enumerate(grp):
                            nc.tensor.matmul(
                                sw[:, i * QT512 : (i + 1) * QT512],
                                kT_g[:, kcol : kcol + KC],
                                qT_sb[:, s * QT512 : (s + 1) * QT512],
                                start=True,
                                stop=True,
                            )
                        pt = wp.tile([128, 1024], BF16, tag="pt")
                        nc.scalar.activation(
                            pt[:, 0:fd],
                            sw[:, 0:fd],
                            mybir.ActivationFunctionType.Exp,
                            scale=float(H) ** -0.5,
                        )
                        for i, s in enumerate(grp):
                            psl = pt[:, i * QT512 : (i + 1) * QT512]
                            j = kc - 8 * s
                            if 0 <= j < 8:
                                o = (7 - j) * 128
                                nc.vector.tensor_mul(
                                    psl, psl, mask_sb[:, o : o + QT512]
                                )
                            nc.tensor.matmul(
                                y_acc[s][0 : H + 1, :],
                                vaug[:, kcb, :],
                                psl,
                                start=(kc == 0),
                                stop=(kc == W_SLOT[s] - 1),
                            )

                # ---- normalize and write out ----
                for s in range(NSLOT):
                    sl = slice(s * QT512, (s + 1) * QT512)
                    rec = wp.tile([1, QT512], F32, tag="rec")
                    nc.vector.reciprocal(rec[:], y_acc[s][H : H + 1, :])
                    bc = sp.tile([H, QT512], F32, tag="swide")
                    nc.tensor.matmul(
                        bc[:], ones_sb[:], rec[:], start=True, stop=True
                    )
                    bc_sb = wp.tile([H, QT512], F32, tag="bcsb")
                    nc.vector.tensor_copy(bc_sb[:], bc[:])
                    nc.vector.tensor_mul(yT_sb[:, sl], y_acc[s][0:H, :], bc_sb[:])

            nc.sync.dma_start(out=out[:], in_=yT_sb[:])

    nc.compile()
    return nc


_NC_CACHE = None


def _get_nc():
    global _NC_CACHE
    if _NC_CACHE is None:
        _NC_CACHE = build_bass()
    return _NC_CACHE


def _make_in_maps(x, Wq, Wk, Wv):
    ident = np.eye(H, dtype=bf16)
    wq16, wk16, wv16 = (w.astype(bf16) for w in (Wq, Wk, Wv))
    p_idx = np.arange(128)[:, None]
    x_idx = np.arange(MASK_W)[None, :]
    masks = [
        (p_idx <= x_idx - off).astype(bf16) for off in (896, 384)
    ]  # rank 0 owns even tiles, rank 1 odd tiles
    in_maps = []
    for c in range(N_CORES):
        b, r = divmod(c, 2)
        rows = np.concatenate(
            [x[b, (2 * s + r) * QT512 : (2 * s + r + 1) * QT512] for s in range(NSLOT)]
        )
        xT_c = np.ascontiguousarray(rows.T).astype(bf16)
        in_maps.append(
            {
                "xT": xT_c,
                "wq": wq16,
                "wk": wk16,
                "wv": wv16,
                "ident": ident,
                "mask": masks[r],
            }
        )
    return in_maps


def _assemble(results):
    y = np.empty((B, T, H), dtype=np.float32)
    for c in range(N_CORES):
        b, r = divmod(c, 2)
        yt = np.asarray(results[c]["out"], dtype=np.float32).T  # [2048, 64]
        for s in range(NSLOT):
            g = 2 * s + r
            y[b, g * QT512 : (g + 1) * QT512] = yt[s * QT512 : (s + 1) * QT512]
    return y


def run(x, Wq, Wk, Wv, trace=False):
    nc = _get_nc()
    in_maps = _make_in_maps(
        np.asarray(x, np.float32),
        np.asarray(Wq, np.float32),
        np.asarray(Wk, np.float32),
        np.asarray(Wv, np.float32),
    )
    res = run_bass_kernel_spmd(nc, in_maps, core_ids=list(range(N_CORES)), trace=trace)
    return _assemble(res.results), res


def kernel(x, Wq, Wk, Wv):
    y, _ = run(x, Wq, Wk, Wv)
    return y


# revision 12
# speedup vs baseline: 1.3819x; 1.3819x over previous
"""Trainium2 Bass kernel for a causal single-head attention block.

Problem: y = softmax(mask(Q K^T / sqrt(H))) V with
  x  [B=4, T=4096, C=1024] f32,  Wq/Wk/Wv [C, H=64] f32.

Sharding (8 NeuronCores): data-parallel over B across core pairs;
within a pair, the T dimension is split by interleaved 512-row tiles
(rank r owns global q-tiles {2s+r}) so the causal work is balanced.
Each core computes Q/K/V for its own 2048 rows, the pair exchanges
K^T and V via an AllGather, and each core runs a flash-attention style
kc-outer loop over its own query rows.

The graph is identical on all 8 cores (SPMD); all rank-dependent
causality is delivered via input *data* (a sliding causal mask sheet).

Layout notes:
 - The host pre-transposes x per core to x^T [C, 2048] bf16 so the
   projections can contract over C on the partition dimension without
   any on-chip transpose of x.
 - Projections produce Q^T/K^T/V^T [64, T] directly (H on partitions),
   which is exactly the operand layout the S^T matmul wants.
 - S^T tiles are [128 k, 512 q]; exp has no running max (logits here
   are ~N(0,1), |s| < ~7, so exp is safe in f32) and the row-sum is
   folded into the PV matmul via a ones-column appended to V.
"""

import numpy as np
import ml_dtypes

import concourse.bass as bass
import concourse.bacc as bacc
import concourse.mybir as mybir
from concourse.tile import TileContext
from concourse.bass_utils import run_bass_kernel_spmd

BF16 = mybir.dt.bfloat16
F32 = mybir.dt.float32
bf16 = ml_dtypes.bfloat16

B, T, C, H = 4, 4096, 1024, 64
N_CORES = 8
TOWN = 2048          # rows owned per core
NSLOT = 4            # q-tiles of 512 rows per core
QT512 = 512
KC = 128             # k-chunk rows
NKC = T // KC        # 32 global k-chunks
W_SLOT = [8, 16, 24, 32]   # uniform kc-window per slot
V_FLAT = TOWN * H    # 131072 elements of V shard in the bounce
CC_K = H * TOWN      # K^T shard elements
CC_V = 128 * 1024    # V shard elements
MASK_W = 896 + 512   # causal mask sheet width


def build_bass():
    nc = bacc.Bacc(
        "TRN2",
        target_bir_lowering=False,
        debug=False,
        enable_asserts=False,
        num_devices=N_CORES,
    )

    xT = nc.declare_dram_parameter("xT", [C, TOWN], BF16, isOutput=False)
    wq = nc.declare_dram_parameter("wq", [C, H], BF16, isOutput=False)
    wk = nc.declare_dram_parameter("wk", [C, H], BF16, isOutput=False)
    wv = nc.declare_dram_parameter("wv", [C, H], BF16, isOutput=False)
    ident = nc.declare_dram_parameter("ident", [H, H], BF16, isOutput=False)
    mask = nc.declare_dram_parameter("mask", [128, MASK_W], BF16, isOutput=False)
    out = nc.declare_dram_parameter("out", [H, TOWN], F32, isOutput=True)

    cc_in_k = nc.dram_tensor("cc_in_k", [CC_K], BF16)
    cc_out_k = nc.dram_tensor("cc_out_k", [2 * CC_K], BF16)
    cc_in_v = nc.dram_tensor("cc_in_v", [CC_V], BF16)
    cc_out_v = nc.dram_tensor("cc_out_v", [2 * CC_V], BF16)

    with TileContext(nc) as tc:
        with (
            tc.tile_pool(name="persist", bufs=1) as pp,
            tc.tile_pool(name="work", bufs=3) as wp,
        ):
            # ---- persistent SBUF tensors ----
            xT_sb = pp.tile([128, 8, TOWN], BF16, tag="xT")
            wq_sb = pp.tile([128, 8, H], BF16, tag="wq")
            wk_sb = pp.tile([128, 8, H], BF16, tag="wk")
            wv_sb = pp.tile([128, 8, H], BF16, tag="wv")
            id_sb = pp.tile([H, H], BF16, tag="ident")
            mask_sb = pp.tile([128, MASK_W], BF16, tag="mask")
            qT_sb = pp.tile([H, TOWN], BF16, tag="qT")
            vT_own = pp.tile([H, TOWN], BF16, tag="vTown")
            kstage = pp.tile([H, TOWN], BF16, tag="kstage")
            vstage = pp.tile([128, 1024], BF16, tag="vstage")
            # rank-blocked columns: [rank0 2048 | rank1 2048]
            kT_g = pp.tile([H, T], BF16, tag="kTg")
            # V rank-blocked [128, kcb, 65]; col 64 of each chunk is ones
            vaug = pp.tile([128, NKC, H + 1], BF16, tag="vaug")
            ones_sb = pp.tile([1, H], F32, tag="ones")
            yT_sb = pp.tile([H, TOWN], F32, tag="yT")

            # ---- loads ----
            nc.sync.dma_start(
                out=xT_sb[:], in_=xT[:].rearrange("(cc p) t -> p cc t", p=128)
            )
            nc.sync.dma_start(
                out=wq_sb[:], in_=wq[:].rearrange("(cc p) h -> p cc h", p=128)
            )
            nc.sync.dma_start(
                out=wk_sb[:], in_=wk[:].rearrange("(cc p) h -> p cc h", p=128)
            )
            nc.sync.dma_start(
                out=wv_sb[:], in_=wv[:].rearrange("(cc p) h -> p cc h", p=128)
            )
            nc.sync.dma_start(out=id_sb[:], in_=ident[:])
            nc.sync.dma_start(out=mask_sb[:], in_=mask[:])
            nc.vector.memset(ones_sb[:], 1.0)
            nc.vector.memset(vaug[:, :, H : H + 1], 1.0)

            # ---- projections: Q^T, K^T, V^T for own rows ----
            with tc.tile_pool(name="proj_ps", bufs=2, space="PSUM") as proj_ps:
                def proj(w_sb, dst, sl):
                    ps = proj_ps.tile([H, QT512], F32, tag="proj", name="ps")
                    for cc in range(8):
                        nc.tensor.matmul(
                            ps[:],
                            w_sb[:, cc, :],
                            xT_sb[:, cc, sl],
                            start=(cc == 0),
                            stop=(cc == 7),
                        )
                    nc.vector.tensor_copy(dst, ps[:])

                # K first so its exchange can start ASAP
                for tt in range(NSLOT):
                    sl = slice(tt * QT512, (tt + 1) * QT512)
                    proj(wk_sb, kstage[:, sl], sl)
                nc.gpsimd.dma_start(
                    out=cc_in_k[:].rearrange("(p t) -> p t", p=H), in_=kstage[:]
                )
                nc.gpsimd.collective_compute(
                    "AllGather",
                    mybir.AluOpType.bypass,
                    replica_groups=[[2 * i, 2 * i + 1] for i in range(N_CORES // 2)],
                    ins=[cc_in_k[:]],
                    outs=[cc_out_k[:]],
                )

                for tt in range(NSLOT):
                    sl = slice(tt * QT512, (tt + 1) * QT512)
                    proj(wv_sb, vT_own[:, sl], sl)
                for tcn in range(16):
                    pst = proj_ps.tile([128, H], BF16, tag="vt")
                    nc.tensor.transpose(
                        pst[:], vT_own[:, tcn * 128 : (tcn + 1) * 128], id_sb[:]
                    )
                    nc.vector.tensor_copy(vstage[:, tcn * H : (tcn + 1) * H], pst[:])
                nc.gpsimd.dma_start(
                    out=cc_in_v[:].rearrange("(p c) -> p c", p=128), in_=vstage[:]
                )
                nc.gpsimd.collective_compute(
                    "AllGather",
                    mybir.AluOpType.bypass,
                    replica_groups=[[2 * i, 2 * i + 1] for i in range(N_CORES // 2)],
                    ins=[cc_in_v[:]],
                    outs=[cc_out_v[:]],
                )

                # Q projection overlaps the collectives
                for tt in range(NSLOT):
                    sl = slice(tt * QT512, (tt + 1) * QT512)
                    proj(wq_sb, qT_sb[:, sl], sl)

            # ---- readback of gathered K^T and V ----
            nc.gpsimd.dma_start(
                out=kT_g[:].rearrange("p (gp t) -> p gp t", gp=2),
                in_=cc_out_k[:].rearrange("(gp p t) -> p gp t", gp=2, p=H),
            )
            blkv = cc_out_v[:].rearrange("(gp p c) -> gp p c", gp=2, p=128)
            for gp in range(2):
                nc.gpsimd.dma_start(
                    out=vaug[:, gp * 16 : (gp + 1) * 16, 0:H],
                    in_=blkv[gp].rearrange("p (tc h) -> p tc h", h=H),
                )

            # ---- attention: kc-outer flash loop ----
            with (
                tc.tile_pool(name="swide", bufs=2, space="PSUM") as sp,
                tc.tile_pool(name="yacc", bufs=1, space="PSUM") as yp,
            ):
                y_acc = [
                    yp.tile([128, QT512], F32, tag=f"y{s}", name=f"y_acc{s}")
                    for s in range(NSLOT)
                ]

                for kc in range(NKC):
                    g = kc // 4
                    kcol = (g % 2) * 2048 + (g // 2) * QT512 + (kc % 4) * KC
                    kcb = (g % 2) * 16 + (g // 2) * 4 + kc % 4
                    smin = kc // 8
                    slots = list(range(smin, NSLOT))
                    for gi in range(0, len(slots), 2):
                        grp = slots[gi : gi + 2]
                        fd = QT512 * len(grp)
                        sw = sp.tile([128, 1024], F32, tag="swide")
                        for i, s in enumerate(grp):
                            nc.tensor.matmul(
                                sw[:, i * QT512 : (i + 1) * QT512],
                                kT_g[:, kcol : kcol + KC],
                                qT_sb[:, s * QT512 : (s + 1) * QT512],
                                start=True,
                                stop=True,
                            )
                        pt = wp.tile([128, 1024], BF16, tag="pt")
                        nc.scalar.activation(
                            pt[:, 0:fd],
                            sw[:, 0:fd],
                            mybir.ActivationFunctionType.Exp,
                            scale=float(H) ** -0.5,
                        )
                        for i, s in enumerate(grp):
                            psl = pt[:, i * QT512 : (i + 1) * QT512]
                            j = kc - 8 * s
                            if 0 <= j < 8:
                                o = (7 - j) * 128
                                nc.vector.tensor_mul(
                                    psl, psl, mask_sb[:, o : o + QT512]
                                )
                            nc.tensor.matmul(
                                y_acc[s][0 : H + 1, :],
                                vaug[:, kcb, :],
                                psl,
                                start=(kc == 0),
                                stop=(kc == W_SLOT[s] - 1),
                            )

                # ---- normalize and write out ----
                lsum = wp.tile([1, NSLOT * QT512], F32, tag="lsum")
                for s in range(NSLOT):
                    nc.vector.tensor_copy(
                        lsum[0:1, s * QT512 : (s + 1) * QT512], y_acc[s][H : H + 1, :]
                    )
                rec = wp.tile([1, NSLOT * QT512], F32, tag="rec")
                nc.vector.reciprocal(rec[:], lsum[:])
                for s in range(NSLOT):
                    sl = slice(s * QT512, (s + 1) * QT512)
                    bc = sp.tile([H, QT512], F32, tag="swide")
                    nc.tensor.matmul(
                        bc[:],
                        ones_sb[:],
                        rec[0:1, s * QT512 : (s + 1) * QT512],
                        start=True,
                        stop=True,
                    )
                    bc_sb = wp.tile([H, QT512], F32, tag="bcsb")
                    nc.vector.tensor_copy(bc_sb[:], bc[:])
                    nc.vector.tensor_mul(yT_sb[:, sl], y_acc[s][0:H, :], bc_sb[:])

            nc.sync.dma_start(out=out[:], in_=yT_sb[:])

    nc.compile()
    return nc


_NC_CACHE = None


def _get_nc():
    global _NC_CACHE
    if _NC_CACHE is None:
        _NC_CACHE = build_bass()
    return _NC_CACHE


def _make_in_maps(x, Wq, Wk, Wv):
    ident = np.eye(H, dtype=bf16)
    wq16, wk16, wv16 = (w.astype(bf16) for w in (Wq, Wk, Wv))
    p_idx = np.arange(128)[:, None]
    x_idx = np.arange(MASK_W)[None, :]
    masks = [
        (p_idx <= x_idx - off).astype(bf16) for off in (896, 384)
    ]  # rank 0 owns even tiles, rank 1 odd tiles
    in_maps = []
    for c in range(N_CORES):
        b, r = divmod(c, 2)
        rows = np.concatenate(
            [x[b, (2 * s + r) * QT512 : (2 * s + r + 1) * QT512] for s in range(NSLOT)]
        )
        xT_c = np.ascontiguousarray(rows.T).astype(bf16)
        in_maps.append(
            {
                "xT": xT_c,
                "wq": wq16,
                "wk": wk16,
                "wv": wv16,
                "ident": ident,
                "mask": masks[r],
            }
        )
    return in_maps


def _assemble(results):
    y = np.empty((B, T, H), dtype=np.float32)
    for c in range(N_CORES):
        b, r = divmod(c, 2)
        yt = np.asarray(results[c]["out"], dtype=np.float32).T  # [2048, 64]
        for s in range(NSLOT):
            g = 2 * s + r
            y[b, g * QT512 : (g + 1) * QT512] = yt[s * QT512 : (s + 1) * QT512]
    return y


def run(x, Wq, Wk, Wv, trace=False):
    nc = _get_nc()
    in_maps = _make_in_maps(
        np.asarray(x, np.float32),
        np.asarray(Wq, np.float32),
        np.asarray(Wk, np.float32),
        np.asarray(Wv, np.float32),
    )
    res = run_bass_kernel_spmd(nc, in_maps, core_ids=list(range(N_CORES)), trace=trace)
    return _assemble(res.results), res


def kernel(x, Wq, Wk, Wv):
    y, _ = run(x, Wq, Wk, Wv)
    return y
